# revision 8
# baseline (speedup 1.0000x reference)
"""Trainium2 Bass kernel for nn_Codec (autoregressive raster-scan codec).

Wavefront decomposition: pixel (ky,kx) of the 122x122 delta grid is computed at
step t = 4*ky + kx (skew-4 anti-diagonal), giving a 606-step sequential chain
with <=31 pixels per image in flight. 8 cores x 3 images each (data-parallel
over the 24 (b,c) pairs); each core runs one 606-step scan with 96 lanes
(3 images x 32 slots, slot = ky mod 32) in the free dimension.

v2: the delta-feature state D (22 rows: 14 dy<=-2 rows, 6 dy=-1 rows, 2 left3
rows) lives in one SBUF tile and is advanced entirely on the PE — one combined
matmul computes [z1_df | z5_df | P@D] into a 118-partition PSUM tile (P = row
shift permutation), and the lag-1/4/8 delta rows are injected as K=1
outer-product matmuls with lane-rotated APs. The new state is copied back to
SBUF by one ACT instruction. No per-step DMAs remain (the v1 SWDGE shift DMAs
were both a ~10us/step descriptor-generation tax and an unsynchronized-read
race, visible as run-to-run nondeterminism).

PSUM layout of the combined tile zc (118 partitions):
  0:48   z1 accumulator      64:88  z5 accumulator      96:118  next D state
MLP: z = W@h matmuls on PE (features on partitions, lanes free), leaky-relu
with per-partition bias on ACT, clip/delta tail on DVE (b7 folded into the
tail via scalar_tensor_tensor). Masked delta rows accumulate into 32-step
batch tiles DMA'd to DRAM; the host unscrambles and computes (loss, invCR).
"""
import sys

sys.path.insert(0, "/opt/trn_rl_repo")
import numpy as np

R = 3
DH = DW = 122
NSTEP = 4 * (DH - 1) + DW  # 606
NL = 96                    # lanes per core = 3 images x 32 slots
CH = 32                    # steps per x-feature chunk
TPAD = 640                 # NSTEP padded to chunk multiple
NBATCH = (NSTEP + 31) // 32  # 19 delta-row batches of 32 steps

_F16 = True               # fp16 matmul operands (PSUM accumulation stays fp32)
_TRACE = False
_TRACE_KW = {}
_LAST_RESULTS = None
_LAST_DALL = None

# ---------------------------------------------------------------- consts layout
_C = {}
_cc = 0


def _span(name, rows, cols):
    global _cc
    _C[name] = (rows, _cc, cols)
    _cc += cols


_span("combo", 22, 118)
_span("w15x", 24, 88)
_span("f47", 1, 118)
_span("f44", 1, 118)
_span("eA6", 1, 118)
_span("eA13", 1, 118)
_span("w2T", 48, 48)
_span("w3T", 48, 48)
_span("w4T", 48, 48)
_span("w5T", 48, 24)
_span("w6T", 24, 12)
_span("w7T", 12, 1)
_span("b7s", 1, 1)
for _i in range(1, 7):
    _span(f"b{_i}", 48, 1)
CC = _cc

# D-state row k -> feature index in the 48-vector
_FEAT = [24 + k for k in range(14)] + [38 + k for k in range(6)] + [45, 46]


def _pack_consts(W):
    """W: dict of weight arrays. Returns (48, CC) float32."""
    c = np.zeros((48, CC), np.float32)

    def put(name, arr):
        rows, c0, cols = _C[name]
        assert arr.shape == (rows, cols), (name, arr.shape)
        c[:rows, c0:c0 + cols] = arr

    W1, W5 = W["W1"], W["W5"]
    combo = np.zeros((22, 118), np.float32)
    for k in range(22):
        combo[k, 0:48] = W1[:, _FEAT[k]]
        combo[k, 64:88] = W5[:, _FEAT[k]]
    # P^T: next-state row m comes from current row k (row shifts within blocks)
    for m in list(range(0, 6)) + list(range(7, 13)) + list(range(14, 19)) + [20]:
        combo[m + 1, 96 + m] = 1.0
    put("combo", combo)

    w15x = np.zeros((24, 88), np.float32)
    w15x[:, 0:48] = W1[:, :24].T
    w15x[:, 64:88] = W5[:, :24].T
    put("w15x", w15x)

    f47 = np.zeros((1, 118), np.float32)
    f47[0, 0:48] = W1[:, 47]
    f47[0, 64:88] = W5[:, 47]
    f47[0, 96 + 21] = 1.0          # rot0 injection -> D row 21
    put("f47", f47)
    f44 = np.zeros((1, 118), np.float32)
    f44[0, 0:48] = W1[:, 44]
    f44[0, 64:88] = W5[:, 44]
    f44[0, 96 + 19] = 1.0          # rot1 injection -> D row 19
    put("f44", f44)
    eA6 = np.zeros((1, 118), np.float32)
    eA6[0, 96 + 6] = 1.0
    put("eA6", eA6)
    eA13 = np.zeros((1, 118), np.float32)
    eA13[0, 96 + 13] = 1.0
    put("eA13", eA13)

    put("w2T", W["W2"].T)
    put("w3T", W["W3"].T)
    put("w4T", W["W4"].T)
    put("w5T", W["W5"].T)
    put("w6T", W["W6"].T)
    put("w7T", W["W7"].T)
    put("b7s", W["b7"][None, :])
    for i in range(1, 7):
        b = W[f"b{i}"]
        put(f"b{i}", np.pad(b[:, None], ((0, 48 - b.shape[0]), (0, 0))))
    return c


def _build_xfeat(xcore):
    """xcore (3,128,128) -> (26, TPAD*96): rows 0-23 x_nb, 24 xc*mask, 25 mask."""
    xf = np.zeros((26, TPAD, NL), np.float32)
    ky, kx = np.meshgrid(np.arange(DH), np.arange(DW), indexing="ij")
    tf = (4 * ky + kx).ravel()
    F = np.empty((DH, DW, 26), np.float32)
    for g in range(3):
        img = xcore[g]
        col = (g * 32 + (ky % 32)).ravel()
        for i in range(3):
            for j in range(7):
                F[:, :, 7 * i + j] = img[i:i + DH, j:j + DW]
        for j in range(3):
            F[:, :, 21 + j] = img[3:3 + DH, j:j + DW]
        F[:, :, 24] = img[3:3 + DH, 3:3 + DW]
        F[:, :, 25] = 1.0
        xf[:, tf, col] = F.reshape(-1, 26).T
    return xf.reshape(26, TPAD * NL)


def _g3(ap):
    return ap.rearrange("p (g c) -> p g c", g=3)


def _build_program():
    import concourse.bass as bass  # noqa: F401
    from concourse import bacc
    import concourse.mybir as mybir
    from concourse.tile import TileContext

    F32 = mybir.dt.float32
    MDT = mybir.dt.float16 if _F16 else mybir.dt.float32
    AF = mybir.ActivationFunctionType
    OP = mybir.AluOpType

    nc = bacc.Bacc(trn_type="TRN2", num_devices=8)
    xfeat_d = nc.dram_tensor("xfeat", [24, TPAD * NL], MDT, kind="ExternalInput")
    xaux_d = nc.dram_tensor("xaux", [2, TPAD * NL], F32, kind="ExternalInput")
    cons_d = nc.dram_tensor("cons", [48, CC], F32, kind="ExternalInput")
    consm_d = nc.dram_tensor("consm", [48, CC], MDT, kind="ExternalInput")
    dstore_d = nc.dram_tensor("dstore", [NBATCH, 32 * NL], MDT, kind="ExternalOutput")

    with TileContext(nc) as tc:
        with tc.tile_pool(name="wp", bufs=1) as wp, \
             tc.tile_pool(name="chp", bufs=3) as chp, \
             tc.tile_pool(name="dfp", bufs=2) as dfp, \
             tc.tile_pool(name="batp", bufs=3) as batp, \
             tc.tile_pool(name="hp", bufs=2) as hp, \
             tc.tile_pool(name="tp", bufs=2) as tp, \
             tc.tile_pool(name="zp", bufs=1, space="PSUM") as zp:

            ct = wp.tile([48, CC], F32)
            nc.sync.dma_start(out=ct, in_=cons_d[:, :])
            ctm = wp.tile([48, CC], MDT)
            nc.sync.dma_start(out=ctm, in_=consm_d[:, :])

            def cs(name, r0=0, rows=None):
                r, c0, cols = _C[name]
                rr = r if rows is None else rows
                return ct[r0:r0 + rr, c0:c0 + cols]

            def cm(name, r0=0, rows=None):
                r, c0, cols = _C[name]
                rr = r if rows is None else rows
                return ctm[r0:r0 + rr, c0:c0 + cols]

            D = dfp.tile([22, NL], MDT, tag="D")
            nc.vector.memset(D[:, :], 0.0)

            chunks = {}
            dm = {}       # t -> (1,96) AP into a batch tile
            batches = {}

            for t in range(NSTEP):
                c = t // CH
                if c not in chunks:
                    ch_t = chp.tile([24, CH * NL], MDT, name="ch", tag="ch")
                    xc_t = chp.tile([1, CH * NL], F32, name="xc", tag="xc")
                    mk_t = chp.tile([1, CH * NL], F32, name="mk", tag="mk")
                    lo, hi = c * CH * NL, (c + 1) * CH * NL
                    nc.sync.dma_start(out=ch_t, in_=xfeat_d[0:24, lo:hi])
                    nc.sync.dma_start(out=xc_t, in_=xaux_d[0:1, lo:hi])
                    nc.sync.dma_start(out=mk_t, in_=xaux_d[1:2, lo:hi])
                    chunks[c] = (ch_t, xc_t, mk_t)
                ch_t, xc_t, mk_t = chunks[c]
                off = (t - c * CH) * NL
                xs = ch_t[0:24, off:off + NL]
                xcm2 = xc_t[0:1, off:off + NL]
                maskr = mk_t[0:1, off:off + NL]

                b = t // 32
                if b not in batches:
                    batches[b] = batp.tile([1, 32 * NL], MDT, name="bat", tag="bat")
                dmv = batches[b][0:1, (t % 32) * NL:(t % 32 + 1) * NL]
                dm[t] = dmv

                # ------- combined preload: [z1_df | z5_df | P@D] + x + fresh -------
                zc = zp.tile([118, NL], F32, tag="zc")
                zcg = _g3(zc[:, :])
                zcD = zc[96:118, :]
                zcDg = _g3(zcD)
                nc.tensor.matmul(zc[:, :], cm("combo"), D[:, :],
                                 start=True, stop=False, skip_group_check=True)
                nc.tensor.matmul(zc[0:88, :], cm("w15x"), xs,
                                 start=False, stop=False, skip_group_check=True)
                if t >= 1:
                    s = _g3(dm[t - 1])
                    nc.tensor.matmul(zc[:, :], cm("f47"), dm[t - 1],
                                     start=False, stop=False, skip_group_check=True)
                    nc.tensor.matmul(zcg[:, :, 1:32], cm("f44"), s[:, :, 0:31],
                                     start=False, stop=False, skip_group_check=True)
                    nc.tensor.matmul(zcg[:, :, 0:1], cm("f44"), s[:, :, 31:32],
                                     start=False, stop=False, skip_group_check=True)
                if t >= 8:
                    s8 = _g3(dm[t - 8])
                    nc.tensor.matmul(zcg[:, :, 3:32], cm("eA6"), s8[:, :, 0:29],
                                     start=False, stop=False, skip_group_check=True)
                    nc.tensor.matmul(zcg[:, :, 0:3], cm("eA6"), s8[:, :, 29:32],
                                     start=False, stop=False, skip_group_check=True)
                if t >= 4:
                    s4 = _g3(dm[t - 4])
                    nc.tensor.matmul(zcg[:, :, 2:32], cm("eA13"), s4[:, :, 0:30],
                                     start=False, stop=False, skip_group_check=True)
                    nc.tensor.matmul(zcg[:, :, 0:2], cm("eA13"), s4[:, :, 30:32],
                                     start=False, stop=False, skip_group_check=True)

                # ---------------- MLP chain ----------------
                h1 = hp.tile([48, NL], MDT, tag="h1")
                nc.scalar.activation(h1[:, :], zc[0:48, :], AF.Lrelu,
                                     bias=cs("b1"), scale=1.0, alpha=0.01)
                # new D state -> SBUF (off the critical chain, after h1 on ACT)
                Dn = dfp.tile([22, NL], MDT, tag="D")
                nc.scalar.activation(Dn[:, :], zcD, AF.Copy, bias=0.0, scale=1.0)

                z2 = zp.tile([48, NL], F32, tag="z2")
                nc.tensor.matmul(z2[:, :], cm("w2T"), h1[:, :], start=True, stop=True)
                h2 = hp.tile([48, NL], MDT, tag="h2")
                nc.scalar.activation(h2[:, :], z2[:, :], AF.Lrelu,
                                     bias=cs("b2"), scale=1.0, alpha=0.01)
                z3 = zp.tile([48, NL], F32, tag="z3")
                nc.tensor.matmul(z3[:, :], cm("w3T"), h2[:, :], start=True, stop=True)
                h3 = hp.tile([48, NL], MDT, tag="h3")
                nc.scalar.activation(h3[:, :], z3[:, :], AF.Lrelu,
                                     bias=cs("b3"), scale=1.0, alpha=0.01)
                z4 = zp.tile([48, NL], F32, tag="z4")
                nc.tensor.matmul(z4[:, :], cm("w4T"), h3[:, :], start=True, stop=True)
                h4 = hp.tile([48, NL], MDT, tag="h4")
                nc.scalar.activation(h4[:, :], z4[:, :], AF.Lrelu,
                                     bias=cs("b4"), scale=1.0, alpha=0.01)
                nc.tensor.matmul(zc[64:88, :], cm("w5T"), h4[:, :],
                                 start=False, stop=True, skip_group_check=True)
                h5 = hp.tile([24, NL], MDT, tag="h5")
                nc.scalar.activation(h5[:, :], zc[64:88, :], AF.Lrelu,
                                     bias=cs("b5", rows=24), scale=1.0, alpha=0.01)
                z6 = zp.tile([12, NL], F32, tag="z6")
                nc.tensor.matmul(z6[:, :], cm("w6T"), h5[:, :], start=True, stop=True)
                h6 = hp.tile([12, NL], MDT, tag="h6")
                nc.scalar.activation(h6[:, :], z6[:, :], AF.Lrelu,
                                     bias=cs("b6", rows=12), scale=1.0, alpha=0.01)
                z7 = zp.tile([1, NL], F32, tag="z7")
                nc.tensor.matmul(z7[:, :], cm("w7T"), h6[:, :], start=True, stop=True)

                # ---------------- tail: dm = xcm2 - clip((z7+b7)*mask) ----------------
                t0 = tp.tile([1, NL], F32, tag="t0")
                nc.vector.scalar_tensor_tensor(out=t0[:, :], in0=z7[:, :],
                                               scalar=cs("b7s"), in1=maskr,
                                               op0=OP.add, op1=OP.mult)
                t1 = tp.tile([1, NL], F32, tag="t1")
                nc.vector.tensor_scalar(out=t1[:, :], in0=t0[:, :], scalar1=1.0,
                                        scalar2=-1.0, op0=OP.min, op1=OP.max)
                nc.vector.tensor_tensor(out=dmv, in0=xcm2, in1=t1[:, :],
                                        op=OP.subtract)

                D = Dn

                if t % 32 == 31 or t == NSTEP - 1:
                    n = (t % 32 + 1) * NL
                    nc.sync.dma_start(out=dstore_d[b:b + 1, 0:n],
                                      in_=batches[b][0:1, 0:n])

    nc.finalize()
    return nc


_PROGRAM = None


def _finalize_outputs(D_all):
    """D_all (8,3,122,122) -> (loss, invCR) matching the reference pipeline."""
    b, ch, h, w = 8, 3, 128, 128
    deltas = np.zeros((b, ch, h - 2, w), np.float32)
    deltas[:, :, R:R + DH, R:R + DW] = D_all
    loss = np.sqrt(np.mean(np.square(deltas), dtype=np.float32), dtype=np.float32)
    de = deltas[:, :, R:, R:-R]
    hist, _ = np.histogram(de, bins=256, range=(-1.0, 1.0))
    prob = hist.astype(np.float32) / np.float32(de.size)
    logp = np.zeros_like(prob)
    np.log2(prob, out=logp, where=prob > 0)
    invCR = np.float32(np.sum(-prob * logp, dtype=np.float32) / 8.0)
    return np.float32(loss), np.float32(invCR)


def kernel(x, W1, b1, W2, b2, W3, b3, W4, b4, W5, b5, W6, b6, W7, b7):
    global _PROGRAM, _LAST_RESULTS, _LAST_DALL
    from concourse.bass_utils import run_bass_kernel_spmd

    x = np.ascontiguousarray(np.asarray(x, np.float32))
    Wd = dict(W1=np.asarray(W1), W2=np.asarray(W2), W3=np.asarray(W3),
              W4=np.asarray(W4), W5=np.asarray(W5), W6=np.asarray(W6),
              W7=np.asarray(W7), b7=np.asarray(b7))
    for i, bb in enumerate([b1, b2, b3, b4, b5, b6], 1):
        Wd[f"b{i}"] = np.asarray(bb)
    cons = _pack_consts(Wd)

    if _PROGRAM is None:
        _PROGRAM = _build_program()
    nc = _PROGRAM

    in_maps = []
    for core in range(8):
        in_maps.append(dict(xfeat=_build_xfeat(x[core]), cons=cons))

    res = run_bass_kernel_spmd(nc, in_maps, core_ids=list(range(8)),
                               trace=_TRACE, **_TRACE_KW)
    _LAST_RESULTS = res

    ky, kx = np.meshgrid(np.arange(DH), np.arange(DW), indexing="ij")
    tg = 4 * ky + kx
    D_all = np.zeros((8, 3, DH, DW), np.float32)
    for core in range(8):
        ds = res.results[core]["dstore"].reshape(-1, NL)  # (608, 96)
        for g in range(3):
            D_all[core, g] = ds[tg, g * 32 + (ky % 32)]
    _LAST_DALL = D_all
    return _finalize_outputs(D_all)


# revision 9
# speedup vs baseline: 1.2091x; 1.2091x over previous
"""Trainium2 Bass kernel for nn_Codec (autoregressive raster-scan codec).

Wavefront decomposition: pixel (ky,kx) of the 122x122 delta grid is computed at
step t = 4*ky + kx (skew-4 anti-diagonal), giving a 606-step sequential chain
with <=31 pixels per image in flight. 8 cores x 3 images each (data-parallel
over the 24 (b,c) pairs); each core runs one 606-step scan with 96 lanes
(3 images x 32 slots, slot = ky mod 32) in the free dimension.

v2: the delta-feature state D (22 rows: 14 dy<=-2 rows, 6 dy=-1 rows, 2 left3
rows) lives in one SBUF tile and is advanced entirely on the PE — one combined
matmul computes [z1_df | z5_df | P@D] into a 118-partition PSUM tile (P = row
shift permutation), and the lag-1/4/8 delta rows are injected as K=1
outer-product matmuls with lane-rotated APs. The new state is copied back to
SBUF by one ACT instruction. No per-step DMAs remain (the v1 SWDGE shift DMAs
were both a ~10us/step descriptor-generation tax and an unsynchronized-read
race, visible as run-to-run nondeterminism).

PSUM layout of the combined tile zc (118 partitions):
  0:48   z1 accumulator      64:88  z5 accumulator      96:118  next D state
MLP: z = W@h matmuls on PE (features on partitions, lanes free), leaky-relu
with per-partition bias on ACT, clip/delta tail on DVE (b7 folded into the
tail via scalar_tensor_tensor). Masked delta rows accumulate into 32-step
batch tiles DMA'd to DRAM; the host unscrambles and computes (loss, invCR).
"""
import sys

sys.path.insert(0, "/opt/trn_rl_repo")
import numpy as np

R = 3
DH = DW = 122
NSTEP = 4 * (DH - 1) + DW  # 606
NL = 96                    # lanes per core = 3 images x 32 slots
CH = 32                    # steps per x-feature chunk
TPAD = 640                 # NSTEP padded to chunk multiple
NBATCH = (NSTEP + 31) // 32  # 19 delta-row batches of 32 steps

_F16 = True               # fp16 matmul operands (PSUM accumulation stays fp32)
_TRACE = False
_TRACE_KW = {}
_LAST_RESULTS = None
_LAST_DALL = None

# ---------------------------------------------------------------- consts layout
_C = {}
_cc = 0


def _span(name, rows, cols):
    global _cc
    _C[name] = (rows, _cc, cols)
    _cc += cols


_span("combo", 22, 118)
_span("w15x", 24, 88)
_span("f47", 1, 118)
_span("f44", 1, 118)
_span("eA6", 1, 118)
_span("eA13", 1, 118)
_span("w2T", 48, 48)
_span("w3T", 48, 48)
_span("w4T", 48, 48)
_span("w5T", 48, 24)
_span("w6T", 24, 12)
_span("w7T", 12, 1)
_span("b7s", 1, 1)
for _i in range(1, 7):
    _span(f"b{_i}", 48, 1)
CC = _cc

# D-state row k -> feature index in the 48-vector
_FEAT = [24 + k for k in range(14)] + [38 + k for k in range(6)] + [45, 46]


def _pack_consts(W):
    """W: dict of weight arrays. Returns (48, CC) float32."""
    c = np.zeros((48, CC), np.float32)

    def put(name, arr):
        rows, c0, cols = _C[name]
        assert arr.shape == (rows, cols), (name, arr.shape)
        c[:rows, c0:c0 + cols] = arr

    W1, W5 = W["W1"], W["W5"]
    combo = np.zeros((22, 118), np.float32)
    for k in range(22):
        combo[k, 0:48] = W1[:, _FEAT[k]]
        combo[k, 64:88] = W5[:, _FEAT[k]]
    # P^T: next-state row m comes from current row k (row shifts within blocks)
    for m in list(range(0, 6)) + list(range(7, 13)) + list(range(14, 19)) + [20]:
        combo[m + 1, 96 + m] = 1.0
    put("combo", combo)

    w15x = np.zeros((24, 88), np.float32)
    w15x[:, 0:48] = W1[:, :24].T
    w15x[:, 64:88] = W5[:, :24].T
    put("w15x", w15x)

    f47 = np.zeros((1, 118), np.float32)
    f47[0, 0:48] = W1[:, 47]
    f47[0, 64:88] = W5[:, 47]
    f47[0, 96 + 21] = 1.0          # rot0 injection -> D row 21
    put("f47", f47)
    f44 = np.zeros((1, 118), np.float32)
    f44[0, 0:48] = W1[:, 44]
    f44[0, 64:88] = W5[:, 44]
    f44[0, 96 + 19] = 1.0          # rot1 injection -> D row 19
    put("f44", f44)
    eA6 = np.zeros((1, 118), np.float32)
    eA6[0, 96 + 6] = 1.0
    put("eA6", eA6)
    eA13 = np.zeros((1, 118), np.float32)
    eA13[0, 96 + 13] = 1.0
    put("eA13", eA13)

    put("w2T", W["W2"].T)
    put("w3T", W["W3"].T)
    put("w4T", W["W4"].T)
    put("w5T", W["W5"].T)
    put("w6T", W["W6"].T)
    put("w7T", W["W7"].T)
    put("b7s", W["b7"][None, :])
    for i in range(1, 7):
        b = W[f"b{i}"]
        put(f"b{i}", np.pad(b[:, None], ((0, 48 - b.shape[0]), (0, 0))))
    return c


def _build_xfeat(xcore):
    """xcore (3,128,128) -> (26, TPAD*96): rows 0-23 x_nb, 24 xc*mask, 25 mask."""
    xf = np.zeros((26, TPAD, NL), np.float32)
    ky, kx = np.meshgrid(np.arange(DH), np.arange(DW), indexing="ij")
    tf = (4 * ky + kx).ravel()
    F = np.empty((DH, DW, 26), np.float32)
    for g in range(3):
        img = xcore[g]
        col = (g * 32 + (ky % 32)).ravel()
        for i in range(3):
            for j in range(7):
                F[:, :, 7 * i + j] = img[i:i + DH, j:j + DW]
        for j in range(3):
            F[:, :, 21 + j] = img[3:3 + DH, j:j + DW]
        F[:, :, 24] = img[3:3 + DH, 3:3 + DW]
        F[:, :, 25] = 1.0
        xf[:, tf, col] = F.reshape(-1, 26).T
    return xf.reshape(26, TPAD * NL)


def _g3(ap):
    return ap.rearrange("p (g c) -> p g c", g=3)


def _build_program():
    import concourse.bass as bass  # noqa: F401
    from concourse import bacc
    import concourse.mybir as mybir
    from concourse.tile import TileContext

    F32 = mybir.dt.float32
    MDT = mybir.dt.float16 if _F16 else mybir.dt.float32
    AF = mybir.ActivationFunctionType
    OP = mybir.AluOpType

    nc = bacc.Bacc(trn_type="TRN2", num_devices=8)
    xfeat_d = nc.dram_tensor("xfeat", [24, TPAD * NL], MDT, kind="ExternalInput")
    xaux_d = nc.dram_tensor("xaux", [2, TPAD * NL], F32, kind="ExternalInput")
    cons_d = nc.dram_tensor("cons", [48, CC], F32, kind="ExternalInput")
    consm_d = nc.dram_tensor("consm", [48, CC], MDT, kind="ExternalInput")
    dstore_d = nc.dram_tensor("dstore", [NBATCH, 32 * NL], MDT, kind="ExternalOutput")

    with TileContext(nc) as tc:
        with tc.tile_pool(name="wp", bufs=1) as wp, \
             tc.tile_pool(name="chp", bufs=3) as chp, \
             tc.tile_pool(name="dfp", bufs=2) as dfp, \
             tc.tile_pool(name="batp", bufs=3) as batp, \
             tc.tile_pool(name="hp", bufs=2) as hp, \
             tc.tile_pool(name="tp", bufs=2) as tp, \
             tc.tile_pool(name="zp", bufs=1, space="PSUM") as zp:

            ct = wp.tile([48, CC], F32)
            nc.sync.dma_start(out=ct, in_=cons_d[:, :])
            ctm = wp.tile([48, CC], MDT)
            nc.sync.dma_start(out=ctm, in_=consm_d[:, :])

            def cs(name, r0=0, rows=None):
                r, c0, cols = _C[name]
                rr = r if rows is None else rows
                return ct[r0:r0 + rr, c0:c0 + cols]

            def cm(name, r0=0, rows=None):
                r, c0, cols = _C[name]
                rr = r if rows is None else rows
                return ctm[r0:r0 + rr, c0:c0 + cols]

            D = dfp.tile([22, NL], MDT, tag="D")
            nc.vector.memset(D[:, :], 0.0)

            chunks = {}
            dm = {}       # t -> (1,96) AP into a batch tile
            batches = {}

            for t in range(NSTEP):
                c = t // CH
                if c not in chunks:
                    ch_t = chp.tile([24, CH * NL], MDT, name="ch", tag="ch")
                    xc_t = chp.tile([1, CH * NL], F32, name="xc", tag="xc")
                    mk_t = chp.tile([1, CH * NL], F32, name="mk", tag="mk")
                    lo, hi = c * CH * NL, (c + 1) * CH * NL
                    nc.sync.dma_start(out=ch_t, in_=xfeat_d[0:24, lo:hi])
                    nc.sync.dma_start(out=xc_t, in_=xaux_d[0:1, lo:hi])
                    nc.sync.dma_start(out=mk_t, in_=xaux_d[1:2, lo:hi])
                    chunks[c] = (ch_t, xc_t, mk_t)
                ch_t, xc_t, mk_t = chunks[c]
                off = (t - c * CH) * NL
                xs = ch_t[0:24, off:off + NL]
                xcm2 = xc_t[0:1, off:off + NL]
                maskr = mk_t[0:1, off:off + NL]

                b = t // 32
                if b not in batches:
                    batches[b] = batp.tile([1, 32 * NL], MDT, name="bat", tag="bat")
                dmv = batches[b][0:1, (t % 32) * NL:(t % 32 + 1) * NL]
                dm[t] = dmv

                # ------- combined preload: [z1_df | z5_df | P@D] + x + fresh -------
                zc = zp.tile([118, NL], F32, tag="zc")
                zcg = _g3(zc[:, :])
                zcD = zc[96:118, :]
                zcDg = _g3(zcD)
                nc.tensor.matmul(zc[:, :], cm("combo"), D[:, :],
                                 start=True, stop=False, skip_group_check=True)
                nc.tensor.matmul(zc[0:88, :], cm("w15x"), xs,
                                 start=False, stop=False, skip_group_check=True)
                if t >= 1:
                    s = _g3(dm[t - 1])
                    nc.tensor.matmul(zc[:, :], cm("f47"), dm[t - 1],
                                     start=False, stop=False, skip_group_check=True)
                    nc.tensor.matmul(zcg[:, :, 1:32], cm("f44"), s[:, :, 0:31],
                                     start=False, stop=False, skip_group_check=True)
                    nc.tensor.matmul(zcg[:, :, 0:1], cm("f44"), s[:, :, 31:32],
                                     start=False, stop=False, skip_group_check=True)
                if t >= 8:
                    s8 = _g3(dm[t - 8])
                    nc.tensor.matmul(zcg[:, :, 3:32], cm("eA6"), s8[:, :, 0:29],
                                     start=False, stop=False, skip_group_check=True)
                    nc.tensor.matmul(zcg[:, :, 0:3], cm("eA6"), s8[:, :, 29:32],
                                     start=False, stop=False, skip_group_check=True)
                if t >= 4:
                    s4 = _g3(dm[t - 4])
                    nc.tensor.matmul(zcg[:, :, 2:32], cm("eA13"), s4[:, :, 0:30],
                                     start=False, stop=False, skip_group_check=True)
                    nc.tensor.matmul(zcg[:, :, 0:2], cm("eA13"), s4[:, :, 30:32],
                                     start=False, stop=False, skip_group_check=True)

                # ---------------- MLP chain ----------------
                h1 = hp.tile([48, NL], MDT, tag="h1")
                nc.scalar.activation(h1[:, :], zc[0:48, :], AF.Lrelu,
                                     bias=cs("b1"), scale=1.0, alpha=0.01)
                # new D state -> SBUF (off the critical chain, after h1 on ACT)
                Dn = dfp.tile([22, NL], MDT, tag="D")
                nc.scalar.activation(Dn[:, :], zcD, AF.Copy, bias=0.0, scale=1.0)

                z2 = zp.tile([48, NL], F32, tag="z2")
                nc.tensor.matmul(z2[:, :], cm("w2T"), h1[:, :], start=True, stop=True)
                h2 = hp.tile([48, NL], MDT, tag="h2")
                nc.scalar.activation(h2[:, :], z2[:, :], AF.Lrelu,
                                     bias=cs("b2"), scale=1.0, alpha=0.01)
                z3 = zp.tile([48, NL], F32, tag="z3")
                nc.tensor.matmul(z3[:, :], cm("w3T"), h2[:, :], start=True, stop=True)
                h3 = hp.tile([48, NL], MDT, tag="h3")
                nc.scalar.activation(h3[:, :], z3[:, :], AF.Lrelu,
                                     bias=cs("b3"), scale=1.0, alpha=0.01)
                z4 = zp.tile([48, NL], F32, tag="z4")
                nc.tensor.matmul(z4[:, :], cm("w4T"), h3[:, :], start=True, stop=True)
                h4 = hp.tile([48, NL], MDT, tag="h4")
                nc.scalar.activation(h4[:, :], z4[:, :], AF.Lrelu,
                                     bias=cs("b4"), scale=1.0, alpha=0.01)
                nc.tensor.matmul(zc[64:88, :], cm("w5T"), h4[:, :],
                                 start=False, stop=True, skip_group_check=True)
                h5 = hp.tile([24, NL], MDT, tag="h5")
                nc.scalar.activation(h5[:, :], zc[64:88, :], AF.Lrelu,
                                     bias=cs("b5", rows=24), scale=1.0, alpha=0.01)
                z6 = zp.tile([12, NL], F32, tag="z6")
                nc.tensor.matmul(z6[:, :], cm("w6T"), h5[:, :], start=True, stop=True)
                h6 = hp.tile([12, NL], MDT, tag="h6")
                nc.scalar.activation(h6[:, :], z6[:, :], AF.Lrelu,
                                     bias=cs("b6", rows=12), scale=1.0, alpha=0.01)
                z7 = zp.tile([1, NL], F32, tag="z7")
                nc.tensor.matmul(z7[:, :], cm("w7T"), h6[:, :], start=True, stop=True)

                # ---------------- tail: dm = xcm2 - clip((z7+b7)*mask) ----------------
                t0 = tp.tile([1, NL], F32, tag="t0")
                nc.vector.scalar_tensor_tensor(out=t0[:, :], in0=z7[:, :],
                                               scalar=cs("b7s"), in1=maskr,
                                               op0=OP.add, op1=OP.mult)
                t1 = tp.tile([1, NL], F32, tag="t1")
                nc.vector.tensor_scalar(out=t1[:, :], in0=t0[:, :], scalar1=1.0,
                                        scalar2=-1.0, op0=OP.min, op1=OP.max)
                nc.vector.tensor_tensor(out=dmv, in0=xcm2, in1=t1[:, :],
                                        op=OP.subtract)

                D = Dn

                if t % 32 == 31 or t == NSTEP - 1:
                    n = (t % 32 + 1) * NL
                    nc.sync.dma_start(out=dstore_d[b:b + 1, 0:n],
                                      in_=batches[b][0:1, 0:n])

    nc.finalize()
    return nc


_PROGRAM = None


def _finalize_outputs(D_all):
    """D_all (8,3,122,122) -> (loss, invCR) matching the reference pipeline."""
    b, ch, h, w = 8, 3, 128, 128
    deltas = np.zeros((b, ch, h - 2, w), np.float32)
    deltas[:, :, R:R + DH, R:R + DW] = D_all
    loss = np.sqrt(np.mean(np.square(deltas), dtype=np.float32), dtype=np.float32)
    de = deltas[:, :, R:, R:-R]
    hist, _ = np.histogram(de, bins=256, range=(-1.0, 1.0))
    prob = hist.astype(np.float32) / np.float32(de.size)
    logp = np.zeros_like(prob)
    np.log2(prob, out=logp, where=prob > 0)
    invCR = np.float32(np.sum(-prob * logp, dtype=np.float32) / 8.0)
    return np.float32(loss), np.float32(invCR)


def kernel(x, W1, b1, W2, b2, W3, b3, W4, b4, W5, b5, W6, b6, W7, b7):
    global _PROGRAM, _LAST_RESULTS, _LAST_DALL
    from concourse.bass_utils import run_bass_kernel_spmd

    x = np.ascontiguousarray(np.asarray(x, np.float32))
    Wd = dict(W1=np.asarray(W1), W2=np.asarray(W2), W3=np.asarray(W3),
              W4=np.asarray(W4), W5=np.asarray(W5), W6=np.asarray(W6),
              W7=np.asarray(W7), b7=np.asarray(b7))
    for i, bb in enumerate([b1, b2, b3, b4, b5, b6], 1):
        Wd[f"b{i}"] = np.asarray(bb)
    cons = _pack_consts(Wd)

    if _PROGRAM is None:
        _PROGRAM = _build_program()
    nc = _PROGRAM

    mdt = np.float16 if _F16 else np.float32
    in_maps = []
    for core in range(8):
        xf = _build_xfeat(x[core])
        in_maps.append(dict(xfeat=xf[:24].astype(mdt), xaux=xf[24:26],
                            cons=cons, consm=cons.astype(mdt)))

    res = run_bass_kernel_spmd(nc, in_maps, core_ids=list(range(8)),
                               trace=_TRACE, **_TRACE_KW)
    _LAST_RESULTS = res

    ky, kx = np.meshgrid(np.arange(DH), np.arange(DW), indexing="ij")
    tg = 4 * ky + kx
    D_all = np.zeros((8, 3, DH, DW), np.float32)
    for core in range(8):
        ds = res.results[core]["dstore"].reshape(-1, NL)  # (608, 96)
        for g in range(3):
            D_all[core, g] = ds[tg, g * 32 + (ky % 32)]
    _LAST_DALL = D_all
    return _finalize_outputs(D_all)
